# revision 22
# baseline (speedup 1.0000x reference)
"""NsNet2 single-step (fc1 + 2x GRU cell + 3x FC) Trainium2 kernel.

Strategy:
  - Pure data parallel: batch B=32768 sharded as 4096 rows per NeuronCore (8 cores).
  - Feature-major layout on chip: activations live as [feat, batch]; host
    transposes inputs/outputs (free; off the HW critical path).
  - ALL matmuls fp8(e4m3)+DoubleRow, fp32 PSUM. Weights are scaled by S=16 on
    the host to lift them out of the fp8 denormal range; the scale is divided
    back out for free via ScalarE activation `scale` or the stt bias slot.
  - fc1 folded into GRU1 input-gate weights (fc1 is linear, f1 only feeds GRU1).
  - z,r gates K-concat their input and hidden operands ([x|h1] resp. [g1|h2])
    and M-concat z|r into one 800-col group (7 chunks).
  - The n-gate hidden matmuls reuse the SAME SBUF operand as z,r via shifted
    chunk views (weights re-laid to match), so only one fp8 operand stream is
    loaded per GRU.
  - 5-stage software pipeline over batch tiles (GRU1 | GRU2 | fc2 | fc3 | fc4)
    so the FIFO Tensor queue never head-of-line blocks on the elementwise
    chain of the same tile.
  - Elementwise work spread over ScalarE (sigmoid/tanh/fc3-relu/fc4-sigmoid),
    VectorE (stt chains, fc2-relu, blend mul/add) and GpSimd (blend sub).
"""

import os
import sys

import numpy as np
import ml_dtypes

sys.path.insert(0, "/opt/trn_rl_repo")

import concourse.bacc as bacc
import concourse.bass as bass
import concourse.mybir as mybir
import concourse.tile as tile
from concourse.bass import ts
from concourse.bass_utils import run_bass_kernel_spmd

BF16 = ml_dtypes.bfloat16
FP8 = ml_dtypes.float8_e4m3

B, F, H, FF = 32768, 257, 400, 600
NCORES = 8
BPC = B // NCORES          # 4096 batch rows per core
Hp, FFp, Fp = 512, 640, 384  # padded feature dims
XHK = 768                  # [x(257) | h1(400) | pad(111)] -> 6 zr K chunks
ZR2K = 896                 # [g1(400) | h2(400) | pad(96)] -> 7 chunks
ZRM = 800                  # contiguous [z(400) | r(400)] output cols -> 7 M chunks
ZRC = 7
NB = 512                   # matmul free-dim tile (one PSUM bank of fp32)
S = 16.0                   # fp8 weight scale (denormal avoidance)

AF = mybir.ActivationFunctionType
ALU = mybir.AluOpType

# packed bias column layout: name -> (offset, n_chunks)
BIAS_LAYOUT = {}
_off = 0
for _n, _c in (("bz1", 4), ("br1", 4), ("bnx1", 4), ("bnh1", 4),
               ("bz2", 4), ("br2", 4), ("bnx2", 4), ("bnh2", 4),
               ("bfc2", 5), ("bfc3", 5), ("bfc4", 3)):
    BIAS_LAYOUT[_n] = (_off, _c)
    _off += _c
BIAS_COLS = _off


def _pad2(a, rows, cols, r0=0):
    out = np.zeros((rows, cols), dtype=np.float64)
    out[r0 : r0 + a.shape[0], : a.shape[1]] = a
    return out


def _bias_tile(vec, padded):
    """Pack a [padded] bias vector as [128, padded//128] fp32 (partition-major)."""
    v = np.zeros(padded, dtype=np.float64)
    v[: vec.shape[0]] = vec
    return np.ascontiguousarray(v.reshape(padded // 128, 128).T).astype(np.float32)


def prepare_weights(inp):
    f64 = {k: np.asarray(v, dtype=np.float64) for k, v in inp.items()}
    w = {}

    # fc1 fold for GRU1 input side
    Wx = {}
    bx = {}
    for name in ("z", "r", "n"):
        Wx[name] = (f64[f"Wi{name}1"] @ f64["Wfc1"]).T          # [F, H]
        bx[name] = f64[f"bi{name}1"] + f64[f"Wi{name}1"] @ f64["bfc1"]

    # GRU1 z,r as separate lane-aligned M=512 groups over K=[x(257)|h1(400)]
    for g, name in (("z", "z"), ("r", "r")):
        Wg = np.zeros((XHK, Hp), dtype=np.float64)
        Wg[:F, :H] = Wx[name]
        Wg[F : F + H, :H] = f64[f"Wh{name}1"].T
        w[f"W{g}1"] = Wg
    # GRU1 n input side: K = xh chunks 0..2 (rows 0..383; rows 257+ are h1 -> 0)
    w["Wn1x"] = _pad2(Wx["n"], Fp, Hp)
    # GRU1 n hidden side: K = xh chunks 2..5 (rows 256..767); h1 lives at 257..656
    w["Wn1h"] = _pad2(f64["Whn1"].T, Hp, Hp, r0=1)

    # GRU2 z,r over K=[g1(400) | h2(400)] (zr2op layout, 896 rows)
    for g in ("z", "r"):
        Wg = np.zeros((ZR2K, Hp), dtype=np.float64)
        Wg[:H, :H] = f64[f"Wi{g}2"].T
        Wg[H : 2 * H, :H] = f64[f"Wh{g}2"].T
        w[f"W{g}2"] = Wg
    # GRU2 n input side: K = zr2op chunks 0..3 (rows 0..511; g1 at 0..399)
    w["Wn2x"] = _pad2(f64["Win2"].T, Hp, Hp)
    # GRU2 n hidden side: K = zr2op chunks 3..6 (rows 384..895); h2 at 400..799
    w["Wn2h"] = _pad2(f64["Whn2"].T, Hp, Hp, r0=16)

    w["Wfc2T"] = _pad2(f64["Wfc2"].T, Hp, FFp)    # [512, 640]
    w["Wfc3T"] = _pad2(f64["Wfc3"].T, FFp, FFp)   # [640, 640]
    w["Wfc4T"] = _pad2(f64["Wfc4"].T, FFp, Fp)    # [640, 384]

    weights = {
        k: np.ascontiguousarray(S * v).astype(FP8) for k, v in w.items()
    }

    parts = [
        ("bz1", _bias_tile(bx["z"] + f64["bhz1"], Hp)),
        ("br1", _bias_tile(bx["r"] + f64["bhr1"], Hp)),
        ("bnx1", _bias_tile(S * bx["n"], Hp)),
        ("bnh1", _bias_tile(S * f64["bhn1"], Hp)),
        ("bz2", _bias_tile(f64["biz2"] + f64["bhz2"], Hp)),
        ("br2", _bias_tile(f64["bir2"] + f64["bhr2"], Hp)),
        ("bnx2", _bias_tile(S * f64["bin2"], Hp)),
        ("bnh2", _bias_tile(S * f64["bhn2"], Hp)),
        ("bfc2", _bias_tile(S * f64["bfc2"], FFp)),
        ("bfc3", _bias_tile(S * f64["bfc3"], FFp)),
        ("bfc4", _bias_tile(f64["bfc4"], Fp)),
    ]
    biases = {"biasT": np.concatenate([p[1] for p in parts], axis=1)}
    return weights, biases


def build_nc(nbt=BPC, nb=NB):
    """Build the per-core Bass program. nbt = per-core batch, nb = free-dim tile."""
    nc = bacc.Bacc("TRN2", target_bir_lowering=False, debug=False)
    bf = mybir.dt.bfloat16
    f32 = mybir.dt.float32
    f8 = mybir.dt.float8e4

    xh8 = nc.declare_dram_parameter("xh8", [XHK, nbt], f8, isOutput=False)
    h1T = nc.declare_dram_parameter("h1T", [Hp, nbt], bf, isOutput=False)
    h2T = nc.declare_dram_parameter("h2T", [Hp, nbt], bf, isOutput=False)
    h28 = nc.declare_dram_parameter("h28", [Hp, nbt], f8, isOutput=False)
    wd = {}
    for name, k, m in (
        ("Wz1", XHK, Hp), ("Wr1", XHK, Hp), ("Wn1x", Fp, Hp), ("Wn1h", Hp, Hp),
        ("Wz2", ZR2K, Hp), ("Wr2", ZR2K, Hp), ("Wn2x", Hp, Hp), ("Wn2h", Hp, Hp),
        ("Wfc2T", Hp, FFp), ("Wfc3T", FFp, FFp), ("Wfc4T", FFp, Fp),
    ):
        wd[name] = nc.declare_dram_parameter(name, [k, m], f8, isOutput=False)
    biasT_d = nc.declare_dram_parameter("biasT", [128, BIAS_COLS], f32, isOutput=False)
    outT = nc.declare_dram_parameter("outT", [Fp, nbt], bf, isOutput=True)

    n_tiles = nbt // nb
    HC = Hp // 128  # 4 M-chunks per gate
    DR = mybir.MatmulPerfMode.DoubleRow

    with tile.TileContext(nc) as tc:
        with (
            tc.tile_pool(name="wpool", bufs=1) as wpool,
            tc.tile_pool(name="bpool", bufs=1) as bpool,
            tc.tile_pool(name="inp2", bufs=2) as inp2,
            tc.tile_pool(name="inp3", bufs=3) as inp3,
            tc.tile_pool(name="inp4", bufs=4) as inp4,
            tc.tile_pool(name="io", bufs=3) as io,
            tc.tile_pool(name="act2", bufs=2) as act2,
            tc.tile_pool(name="act3", bufs=3) as act3,
            tc.tile_pool(name="act4", bufs=4) as act4,
            tc.tile_pool(name="psum", bufs=2, space="PSUM") as psum,
        ):
            # ACT-table warmup: first ScalarE transcendental carries the
            # ACT_TABLE_LOAD pseudo-inst; keep it off the critical chain.
            warm = bpool.tile([128, 1], f32, tag="warm")
            nc.vector.memset(warm, 0.0)
            nc.scalar.activation(warm, warm, AF.Sigmoid)

            W, BT = {}, {}

            def load_w(name, eng):
                dram = wd[name]
                k, m = dram.shape
                t = wpool.tile([128, k // 128, m], dram.dtype, tag=name)
                r = dram.rearrange("(c p) m -> p c m", p=128)
                for c in range(k // 128):
                    eng.dma_start(out=t[:, c, :], in_=r[:, c, :])
                W[name] = t

            def load_bias():
                biasT = bpool.tile([128, BIAS_COLS], f32, tag="biasT")
                nc.sync.dma_start(out=biasT, in_=biasT_d[:, :])
                for _n, (_o, _c) in BIAS_LAYOUT.items():
                    BT[_n] = biasT[:, _o : _o + _c]

            xh_r = xh8.rearrange("(c p) n -> p c n", p=128)
            h1_bl = h1T.rearrange("(c p) n -> p c n", p=128)
            h2_bl = h2T.rearrange("(c p) n -> p c n", p=128)
            h2_s0 = h28[0:112, :]                     # -> partitions 16..127 of zr2 chunk 3
            h2_s1 = h28[112:496, :].rearrange("(c p) n -> p c n", p=128)
            outT_r = outT.rearrange("(c p) n -> p c n", p=128)

            ST = [dict() for _ in range(n_tiles)]

            def load_inputs(t):
                sl = ts(t, nb)
                xh = inp2.tile([128, 6, nb], f8, tag="xh")      # zr1/nx1/nh1 K operand
                nc.sync.dma_start(out=xh, in_=xh_r[:, :, sl])
                h1s = inp3.tile([128, HC, nb], bf, tag="h1s")   # blend h1
                nc.sync.dma_start(out=h1s, in_=h1_bl[:, :, sl])
                h2s = inp4.tile([128, HC, nb], bf, tag="h2s")   # blend h2
                nc.sync.dma_start(out=h2s, in_=h2_bl[:, :, sl])
                ST[t]["xh"], ST[t]["h1s"], ST[t]["h2s"] = xh, h1s, h2s

            def matseq_dr(ps, Wt, kc, col, mw, rhs_t, r0=0):
                """fp8 accumulation over kc K-chunks using DoubleRow on
                consecutive chunk pairs (odd tail chunk = normal). rhs chunks
                start at r0 within the operand tile."""
                n = (kc + 1) // 2
                for i in range(n):
                    k = 2 * i
                    if k + 2 <= kc:
                        nc.tensor.matmul(
                            ps, Wt[:, k : k + 2, col : col + mw],
                            rhs_t[:, r0 + k : r0 + k + 2, :],
                            start=(i == 0), stop=(i == n - 1), perf_mode=DR)
                    else:
                        nc.tensor.matmul(
                            ps, Wt[:, k, col : col + mw], rhs_t[:, r0 + k, :],
                            start=(i == 0), stop=(i == n - 1))

            def gru_mm(rhs, Wz, Wr, zr_kc, Wnx, nx_kc, Wnh, nh_r0,
                       bz, br, bnx, bnh):
                """Matmul + activation-chain part of one GRU step. Groups are
                interleaved per feature chunk m ([Z R NX NH] x 4) so each PSUM
                tag is reused only every 4th group and consumers keep up.
                Returns (z_t, n_t); the h' blend is issued separately so the
                VectorE queue is not head-of-line blocked on late tanh results.

                All chain ops run full-lane: pad lanes of every operand are
                exact zeros (zero weights -> zero psum, zero bias), except
                z/r pads which are sigmoid(0)=0.5 and get multiplied by the
                zero pads of the other blend operand."""
                z_t = act4.tile([128, HC, nb], bf, tag="z_t")
                r_t = act3.tile([128, HC, nb], bf, tag="r_t")
                npre = act2.tile([128, HC, nb], f32, tag="npre")
                for m in range(HC):
                    col = m * 128
                    psz = psum.tile([128, nb], f32, tag="ps_zr")
                    matseq_dr(psz, Wz, zr_kc, col, 128, rhs)
                    nc.scalar.activation(z_t[:, m, :], psz, AF.Sigmoid,
                                         bias=bz[:, m : m + 1], scale=1.0 / S)
                    psr = psum.tile([128, nb], f32, tag="ps_zr")
                    matseq_dr(psr, Wr, zr_kc, col, 128, rhs)
                    nc.scalar.activation(r_t[:, m, :], psr, AF.Sigmoid,
                                         bias=br[:, m : m + 1], scale=1.0 / S)
                    psx = psum.tile([128, nb], f32, tag="ps_nx")
                    matseq_dr(psx, Wnx, nx_kc, col, 128, rhs)
                    psh = psum.tile([128, nb], f32, tag="ps_nh")
                    matseq_dr(psh, Wnh, HC, col, 128, rhs, r0=nh_r0)
                    # rhn = (psh + bnh) * r ; npre = (psx + bnx) + rhn
                    rhn = act3.tile([128, nb], f32, tag="rhn")
                    nc.vector.scalar_tensor_tensor(
                        rhn, psh, bnh[:, m : m + 1],
                        r_t[:, m, :], op0=ALU.add, op1=ALU.mult)
                    nc.vector.scalar_tensor_tensor(
                        npre[:, m, :], psx, bnx[:, m : m + 1],
                        rhn, op0=ALU.add, op1=ALU.add)
                    # one deferred blend op from the previous iteration rides
                    # along per chunk: its deps are long satisfied, so it is
                    # pure VectorE throughput here and never blocks the stt's
                    # behind it.
                    if fillers:
                        fillers.popleft()()
                # n = tanh(npre/S), one fat ACT over all 4 chunks
                n_t = act4.tile([128, HC, nb], bf, tag="n_t")
                nc.scalar.activation(n_t, npre, AF.Tanh, scale=1.0 / S)
                return z_t, n_t

            def blend(z_t, n_t, h_bl, out, full):
                """h' = n + z*(h - n) as 4 slot-sized closures interleaved into
                the next iteration's chunk loops. Chunk pairs are staggered so
                each closure's inputs are ready ~2us before it is reached: the
                GpSimd half-sub issued in slot 0 is consumed only in slot 2.
                full=True writes all 4 chunks of `out` (pads compute to zero);
                full=False (zr2op target) writes chunks 0..2 + rows 0..15 of
                chunk 3 so the h2 half of the operand is preserved."""
                d = act3.tile([128, HC, nb], bf, tag="d")
                zd = act3.tile([128, HC, nb], bf, tag="zd")

                def s0():
                    nc.gpsimd.tensor_sub(d[:, 0:2, :], h_bl[:, 0:2, :], n_t[:, 0:2, :])
                    nc.vector.tensor_sub(d[:, 2:4, :], h_bl[:, 2:4, :], n_t[:, 2:4, :])

                def s1():
                    nc.vector.tensor_mul(zd[:, 2:4, :], z_t[:, 2:4, :], d[:, 2:4, :])

                def s2():
                    if full:
                        nc.vector.tensor_add(out[:, 2:4, :], n_t[:, 2:4, :], zd[:, 2:4, :])
                    else:
                        nc.vector.tensor_add(out[:, 2, :], n_t[:, 2, :], zd[:, 2, :])
                        nc.vector.tensor_add(out[0:16, 3, :], n_t[0:16, 3, :],
                                             zd[0:16, 3, :])
                    nc.vector.tensor_mul(zd[:, 0:2, :], z_t[:, 0:2, :], d[:, 0:2, :])

                def s3():
                    nc.vector.tensor_add(out[:, 0:2, :], n_t[:, 0:2, :], zd[:, 0:2, :])

                return [s0, s1, s2, s3]

            def stage_gru1(t):
                sl = ts(t, nb)
                # zr2op = GRU2's [g1|h2] fp8 operand: g1 written by GRU1 blend,
                # h2 stitched in by DMA at concat rows 400..799.
                zr2op = io.tile([128, ZRC, nb], f8, tag="zr2op")
                nc.sync.dma_start(out=zr2op[16:128, 3, :], in_=h2_s0[:, sl])
                nc.sync.dma_start(out=zr2op[:, 4:7, :], in_=h2_s1[:, :, sl])
                ST[t]["zr2op"] = zr2op
                z_t, n_t = gru_mm(ST[t]["xh"], W["Wz1"], W["Wr1"], 6,
                                  W["Wn1x"], 3, W["Wn1h"], 2,
                                  BT["bz1"], BT["br1"], BT["bnx1"], BT["bnh1"])
                return blend(z_t, n_t, ST[t]["h1s"], zr2op, full=False)

            def stage_gru2(t):
                zr2op = ST[t]["zr2op"]
                g2 = io.tile([128, HC, nb], f8, tag="g2")
                z_t, n_t = gru_mm(zr2op, W["Wz2"], W["Wr2"], ZRC,
                                  W["Wn2x"], HC, W["Wn2h"], 3,
                                  BT["bz2"], BT["br2"], BT["bnx2"], BT["bnh2"])
                ST[t]["g2"] = g2
                return blend(z_t, n_t, ST[t]["h2s"], g2, full=True)

            def stage_fc2(t):
                g2 = ST[t].pop("g2")
                f2 = io.tile([128, FFp // 128, nb], f8, tag="f2")
                for m in range(FFp // 128):
                    ps = psum.tile([128, nb], f32, tag="ps_fc")
                    matseq_dr(ps, W["Wfc2T"], HC, m * 128, 128, g2)
                    # f2 = relu(ps + S*b) = S*relu(W g2 + b)
                    nc.vector.tensor_scalar(
                        f2[:, m, :], ps, BT["bfc2"][:, m : m + 1], 0.0,
                        op0=ALU.add, op1=ALU.max)
                ST[t]["f2"] = f2

            def stage_fc3(t):
                f2 = ST[t].pop("f2")
                f3 = io.tile([128, FFp // 128, nb], f8, tag="f3")
                for m in range(FFp // 128):
                    ps = psum.tile([128, nb], f32, tag="ps_fc")
                    matseq_dr(ps, W["Wfc3T"], FFp // 128, m * 128, 128, f2)
                    # f3 = relu(ps/S + S*b) = S*relu(W f2 + b)
                    nc.scalar.activation(f3[:, m, :], ps, AF.Relu,
                                         bias=BT["bfc3"][:, m : m + 1],
                                         scale=1.0 / S)
                ST[t]["f3"] = f3

            def stage_fc4(t):
                sl = ts(t, nb)
                f3 = ST[t].pop("f3")
                o = io.tile([128, Fp // 128, nb], bf, tag="o")
                for m in range(Fp // 128):
                    ps = psum.tile([128, nb], f32, tag="ps_fc")
                    matseq_dr(ps, W["Wfc4T"], FFp // 128, m * 128, 128, f3)
                    nc.scalar.activation(o[:, m, :], ps, AF.Sigmoid,
                                         bias=BT["bfc4"][:, m : m + 1],
                                         scale=1.0 / (S * S))
                nc.sync.dma_start(out=outT_r[:, :, sl], in_=o)

            # startup loads: sync carries GRU1 weights + first inputs; GRU2
            # weights go on the (otherwise idle at start) gpsimd ring; FC
            # weights on the scalar ring (needed only from iteration 2 on).
            # Startup: only what iteration 0 needs loads immediately; GRU2 and
            # FC weights are pushed past the critical window so they do not
            # steal HBM bandwidth from xh/Wz1/Wr1 (first-matmul gating).
            load_inputs(0)
            load_w("Wz1", nc.sync)
            load_bias()
            load_w("Wr1", nc.sync)
            load_w("Wn1x", nc.sync)
            load_w("Wn1h", nc.sync)
            with tc.tile_wait_until(0.004):
                for name in ("Wz2", "Wr2", "Wn2x", "Wn2h"):
                    load_w(name, nc.gpsimd)
            with tc.tile_wait_until(0.009):
                for name in ("Wfc2T", "Wfc3T", "Wfc4T"):
                    load_w(name, nc.scalar)

            from collections import deque
            fillers = deque()
            PF = 1  # input prefetch depth (iterations ahead)
            for i in range(n_tiles + 4):
                nxt = []
                if i < n_tiles:
                    if i + PF < n_tiles:
                        load_inputs(i + PF)
                    nxt += stage_gru1(i)
                else:
                    # no A-stage to consume pending blends: flush them now so
                    # the B-stage below never reads a zr2op whose writes are
                    # still queued behind its own chain (circular wait).
                    while fillers:
                        fillers.popleft()()
                if 0 <= i - 1 < n_tiles:
                    nxt += stage_gru2(i - 1)
                if 0 <= i - 3 < n_tiles:
                    stage_fc3(i - 3)
                if 0 <= i - 4 < n_tiles:
                    stage_fc4(i - 4)
                if 0 <= i - 2 < n_tiles:
                    stage_fc2(i - 2)
                while fillers:
                    fillers.popleft()()
                fillers.extend(nxt)

    nc.compile()
    return nc


def _shard_inputs(inp, weights, biases):
    x = np.asarray(inp["x"], dtype=np.float32)
    h1 = np.asarray(inp["h1"], dtype=np.float32)
    h2 = np.asarray(inp["h2"], dtype=np.float32)

    xh8 = np.zeros((NCORES, XHK, BPC), dtype=FP8)    # matmul operand [x|h1]
    h1T = np.zeros((NCORES, Hp, BPC), dtype=BF16)    # blend h1
    h2T = np.zeros((NCORES, Hp, BPC), dtype=BF16)    # blend h2
    h28 = np.zeros((NCORES, Hp, BPC), dtype=FP8)     # zr2op h2 stitch source
    for i in range(NCORES):
        sl = slice(i * BPC, (i + 1) * BPC)
        xh8[i, :F] = x[sl].T.astype(FP8)
        xh8[i, F : F + H] = h1[sl].T.astype(FP8)
        h1T[i, :H] = h1[sl].T.astype(BF16)
        h2T[i, :H] = h2[sl].T.astype(BF16)
        h28[i, :H] = h2[sl].T.astype(FP8)

    in_maps = []
    for i in range(NCORES):
        m = {"xh8": xh8[i], "h1T": h1T[i], "h2T": h2T[i], "h28": h28[i]}
        m.update(weights)
        m.update(biases)
        in_maps.append(m)
    return in_maps


def _run(inp, trace=False):
    weights, biases = prepare_weights(inp)
    nc = build_nc()
    in_maps = _shard_inputs(inp, weights, biases)
    res = run_bass_kernel_spmd(nc, in_maps, list(range(NCORES)), trace=trace)
    out = np.empty((B, F), dtype=np.float32)
    for i in range(NCORES):
        out[i * BPC : (i + 1) * BPC] = (
            np.asarray(res.results[i]["outT"][:F]).astype(np.float32).T
        )
    return out, res


def kernel(**inputs) -> np.ndarray:
    out, _ = _run(inputs, trace=False)
    return out


# revision 24
# speedup vs baseline: 1.0356x; 1.0356x over previous
"""NsNet2 single-step (fc1 + 2x GRU cell + 3x FC) Trainium2 kernel.

Strategy:
  - Pure data parallel: batch B=32768 sharded as 4096 rows per NeuronCore (8 cores).
  - Feature-major layout on chip: activations live as [feat, batch]; host
    transposes inputs/outputs (free; off the HW critical path).
  - ALL matmuls fp8(e4m3)+DoubleRow, fp32 PSUM. Weights are scaled by S=16 on
    the host to lift them out of the fp8 denormal range; the scale is divided
    back out for free via ScalarE activation `scale` or the stt bias slot.
  - fc1 folded into GRU1 input-gate weights (fc1 is linear, f1 only feeds GRU1).
  - z,r gates K-concat their input and hidden operands ([x|h1] resp. [g1|h2])
    and M-concat z|r into one 800-col group (7 chunks).
  - The n-gate hidden matmuls reuse the SAME SBUF operand as z,r via shifted
    chunk views (weights re-laid to match), so only one fp8 operand stream is
    loaded per GRU.
  - 5-stage software pipeline over batch tiles (GRU1 | GRU2 | fc2 | fc3 | fc4)
    so the FIFO Tensor queue never head-of-line blocks on the elementwise
    chain of the same tile.
  - Elementwise work spread over ScalarE (sigmoid/tanh/fc3-relu/fc4-sigmoid),
    VectorE (stt chains, fc2-relu, blend mul/add) and GpSimd (blend sub).
"""

import os
import sys

import numpy as np
import ml_dtypes

sys.path.insert(0, "/opt/trn_rl_repo")

import concourse.bacc as bacc
import concourse.bass as bass
import concourse.mybir as mybir
import concourse.tile as tile
from concourse.bass import ts
from concourse.bass_utils import run_bass_kernel_spmd

BF16 = ml_dtypes.bfloat16
FP8 = ml_dtypes.float8_e4m3

B, F, H, FF = 32768, 257, 400, 600
NCORES = 8
BPC = B // NCORES          # 4096 batch rows per core
Hp, FFp, Fp = 512, 640, 384  # padded feature dims
XHK = 768                  # [x(257) | h1(400) | pad(111)] -> 6 zr K chunks
ZR2K = 896                 # [g1(400) | h2(400) | pad(96)] -> 7 chunks
ZRM = 800                  # contiguous [z(400) | r(400)] output cols -> 7 M chunks
ZRC = 7
NB = 512                   # matmul free-dim tile (one PSUM bank of fp32)
S = 16.0                   # fp8 weight scale (denormal avoidance)

AF = mybir.ActivationFunctionType
ALU = mybir.AluOpType

# packed bias column layout: name -> (offset, n_chunks)
BIAS_LAYOUT = {}
_off = 0
for _n, _c in (("bz1", 4), ("br1", 4), ("bnx1", 4), ("bnh1", 4),
               ("bz2", 4), ("br2", 4), ("bnx2", 4), ("bnh2", 4),
               ("bfc2", 5), ("bfc3", 5), ("bfc4", 3)):
    BIAS_LAYOUT[_n] = (_off, _c)
    _off += _c
BIAS_COLS = _off


def _pad2(a, rows, cols, r0=0):
    out = np.zeros((rows, cols), dtype=np.float64)
    out[r0 : r0 + a.shape[0], : a.shape[1]] = a
    return out


def _bias_tile(vec, padded):
    """Pack a [padded] bias vector as [128, padded//128] fp32 (partition-major)."""
    v = np.zeros(padded, dtype=np.float64)
    v[: vec.shape[0]] = vec
    return np.ascontiguousarray(v.reshape(padded // 128, 128).T).astype(np.float32)


def prepare_weights(inp):
    f64 = {k: np.asarray(v, dtype=np.float64) for k, v in inp.items()}
    w = {}

    # fc1 fold for GRU1 input side
    Wx = {}
    bx = {}
    for name in ("z", "r", "n"):
        Wx[name] = (f64[f"Wi{name}1"] @ f64["Wfc1"]).T          # [F, H]
        bx[name] = f64[f"bi{name}1"] + f64[f"Wi{name}1"] @ f64["bfc1"]

    # GRU1 z,r as separate lane-aligned M=512 groups over K=[x(257)|h1(400)]
    for g, name in (("z", "z"), ("r", "r")):
        Wg = np.zeros((XHK, Hp), dtype=np.float64)
        Wg[:F, :H] = Wx[name]
        Wg[F : F + H, :H] = f64[f"Wh{name}1"].T
        w[f"W{g}1"] = Wg
    # GRU1 n input side: K = xh chunks 0..1 (rows 0..255) only; the x[256]
    # contribution is folded into Wn1h's row 0 (= xh row 256), saving a pass.
    w["Wn1x"] = _pad2(Wx["n"][:256], 256, Hp)
    # GRU1 n hidden side: K = xh chunks 2..5 (rows 256..767); h1 lives at 257..656
    Wn1h = _pad2(f64["Whn1"].T, Hp, Hp, r0=1)
    Wn1h[0, :H] = Wx["n"][256]
    w["Wn1h"] = Wn1h

    # GRU2 z,r over K=[g1(400) | h2(400)] (zr2op layout, 896 rows)
    for g in ("z", "r"):
        Wg = np.zeros((ZR2K, Hp), dtype=np.float64)
        Wg[:H, :H] = f64[f"Wi{g}2"].T
        Wg[H : 2 * H, :H] = f64[f"Wh{g}2"].T
        w[f"W{g}2"] = Wg
    # GRU2 n input side: K = zr2op chunks 0..3 (rows 0..511; g1 at 0..399)
    w["Wn2x"] = _pad2(f64["Win2"].T, Hp, Hp)
    # GRU2 n hidden side: K = zr2op chunks 3..6 (rows 384..895); h2 at 400..799
    w["Wn2h"] = _pad2(f64["Whn2"].T, Hp, Hp, r0=16)

    w["Wfc2T"] = _pad2(f64["Wfc2"].T, Hp, FFp)    # [512, 640]
    w["Wfc3T"] = _pad2(f64["Wfc3"].T, FFp, FFp)   # [640, 640]
    w["Wfc4T"] = _pad2(f64["Wfc4"].T, FFp, Fp)    # [640, 384]

    weights = {
        k: np.ascontiguousarray(S * v).astype(FP8) for k, v in w.items()
    }

    parts = [
        ("bz1", _bias_tile(bx["z"] + f64["bhz1"], Hp)),
        ("br1", _bias_tile(bx["r"] + f64["bhr1"], Hp)),
        ("bnx1", _bias_tile(S * bx["n"], Hp)),
        ("bnh1", _bias_tile(S * f64["bhn1"], Hp)),
        ("bz2", _bias_tile(f64["biz2"] + f64["bhz2"], Hp)),
        ("br2", _bias_tile(f64["bir2"] + f64["bhr2"], Hp)),
        ("bnx2", _bias_tile(S * f64["bin2"], Hp)),
        ("bnh2", _bias_tile(S * f64["bhn2"], Hp)),
        ("bfc2", _bias_tile(S * f64["bfc2"], FFp)),
        ("bfc3", _bias_tile(S * f64["bfc3"], FFp)),
        ("bfc4", _bias_tile(f64["bfc4"], Fp)),
    ]
    biases = {"biasT": np.concatenate([p[1] for p in parts], axis=1)}
    return weights, biases


def build_nc(nbt=BPC, nb=NB):
    """Build the per-core Bass program. nbt = per-core batch, nb = free-dim tile."""
    nc = bacc.Bacc("TRN2", target_bir_lowering=False, debug=False)
    bf = mybir.dt.bfloat16
    f32 = mybir.dt.float32
    f8 = mybir.dt.float8e4

    xh8 = nc.declare_dram_parameter("xh8", [XHK, nbt], f8, isOutput=False)
    h1T = nc.declare_dram_parameter("h1T", [Hp, nbt], bf, isOutput=False)
    h2T = nc.declare_dram_parameter("h2T", [Hp, nbt], bf, isOutput=False)
    h28 = nc.declare_dram_parameter("h28", [Hp, nbt], f8, isOutput=False)
    wd = {}
    for name, k, m in (
        ("Wz1", XHK, Hp), ("Wr1", XHK, Hp), ("Wn1x", 256, Hp), ("Wn1h", Hp, Hp),
        ("Wz2", ZR2K, Hp), ("Wr2", ZR2K, Hp), ("Wn2x", Hp, Hp), ("Wn2h", Hp, Hp),
        ("Wfc2T", Hp, FFp), ("Wfc3T", FFp, FFp), ("Wfc4T", FFp, Fp),
    ):
        wd[name] = nc.declare_dram_parameter(name, [k, m], f8, isOutput=False)
    biasT_d = nc.declare_dram_parameter("biasT", [128, BIAS_COLS], f32, isOutput=False)
    outT = nc.declare_dram_parameter("outT", [Fp, nbt], bf, isOutput=True)

    n_tiles = nbt // nb
    HC = Hp // 128  # 4 M-chunks per gate
    DR = mybir.MatmulPerfMode.DoubleRow

    with tile.TileContext(nc) as tc:
        with (
            tc.tile_pool(name="wpool", bufs=1) as wpool,
            tc.tile_pool(name="bpool", bufs=1) as bpool,
            tc.tile_pool(name="inp2", bufs=2) as inp2,
            tc.tile_pool(name="inp3", bufs=3) as inp3,
            tc.tile_pool(name="inp4", bufs=4) as inp4,
            tc.tile_pool(name="io", bufs=3) as io,
            tc.tile_pool(name="act2", bufs=2) as act2,
            tc.tile_pool(name="act3", bufs=3) as act3,
            tc.tile_pool(name="act4", bufs=4) as act4,
            tc.tile_pool(name="psum", bufs=8, space="PSUM") as psum,
        ):
            # ACT-table warmup: first ScalarE transcendental carries the
            # ACT_TABLE_LOAD pseudo-inst; keep it off the critical chain.
            warm = bpool.tile([128, 1], f32, tag="warm")
            nc.vector.memset(warm, 0.0)
            nc.scalar.activation(warm, warm, AF.Sigmoid)

            W, BT = {}, {}

            def load_w(name, eng, eng2=None):
                dram = wd[name]
                k, m = dram.shape
                t = wpool.tile([128, k // 128, m], dram.dtype, tag=name)
                r = dram.rearrange("(c p) m -> p c m", p=128)
                for c in range(k // 128):
                    e = eng2 if (eng2 is not None and c % 2) else eng
                    e.dma_start(out=t[:, c, :], in_=r[:, c, :])
                W[name] = t

            def load_bias():
                biasT = bpool.tile([128, BIAS_COLS], f32, tag="biasT")
                nc.sync.dma_start(out=biasT, in_=biasT_d[:, :])
                for _n, (_o, _c) in BIAS_LAYOUT.items():
                    BT[_n] = biasT[:, _o : _o + _c]

            xh_r = xh8.rearrange("(c p) n -> p c n", p=128)
            h1_bl = h1T.rearrange("(c p) n -> p c n", p=128)
            h2_bl = h2T.rearrange("(c p) n -> p c n", p=128)
            h2_s0 = h28[0:112, :]                     # -> partitions 16..127 of zr2 chunk 3
            h2_s1 = h28[112:496, :].rearrange("(c p) n -> p c n", p=128)
            outT_r = outT.rearrange("(c p) n -> p c n", p=128)

            ST = [dict() for _ in range(n_tiles)]

            def load_inputs(t):
                sl = ts(t, nb)
                xh = inp2.tile([128, 6, nb], f8, tag="xh")      # zr1/nx1/nh1 K operand
                nc.sync.dma_start(out=xh, in_=xh_r[:, :, sl])
                ST[t]["xh"] = xh
                load_blend_inputs(t)

            def load_blend_inputs(t):
                sl = ts(t, nb)
                h1s = inp3.tile([128, HC, nb], bf, tag="h1s")   # blend h1
                nc.sync.dma_start(out=h1s, in_=h1_bl[:, :, sl])
                h2s = inp4.tile([128, HC, nb], bf, tag="h2s")   # blend h2
                nc.sync.dma_start(out=h2s, in_=h2_bl[:, :, sl])
                ST[t]["h1s"], ST[t]["h2s"] = h1s, h2s

            def load_first_inputs():
                sl = ts(0, nb)
                xh = inp2.tile([128, 6, nb], f8, tag="xh")
                nc.sync.dma_start(out=xh, in_=xh_r[:, :, sl])
                ST[0]["xh"] = xh

            def matseq_dr(ps, Wt, kc, col, mw, rhs_t, r0=0):
                """fp8 accumulation over kc K-chunks using DoubleRow on
                consecutive chunk pairs (odd tail chunk = normal). rhs chunks
                start at r0 within the operand tile."""
                n = (kc + 1) // 2
                for i in range(n):
                    k = 2 * i
                    if k + 2 <= kc:
                        nc.tensor.matmul(
                            ps, Wt[:, k : k + 2, col : col + mw],
                            rhs_t[:, r0 + k : r0 + k + 2, :],
                            start=(i == 0), stop=(i == n - 1), perf_mode=DR)
                    else:
                        nc.tensor.matmul(
                            ps, Wt[:, k, col : col + mw], rhs_t[:, r0 + k, :],
                            start=(i == 0), stop=(i == n - 1))

            def gru_mm(rhs, Wz, Wr, zr_kc, Wnx, nx_kc, Wnh, nh_r0,
                       bz, br, bnx, bnh):
                """Matmul + activation-chain part of one GRU step. Groups are
                interleaved per feature chunk m ([Z R NX NH] x 4) so each PSUM
                tag is reused only every 4th group and consumers keep up.
                Returns (z_t, n_t); the h' blend is issued separately so the
                VectorE queue is not head-of-line blocked on late tanh results.

                All chain ops run full-lane: pad lanes of every operand are
                exact zeros (zero weights -> zero psum, zero bias), except
                z/r pads which are sigmoid(0)=0.5 and get multiplied by the
                zero pads of the other blend operand."""
                z_t = act4.tile([128, HC, nb], bf, tag="z_t")
                r_t = act3.tile([128, HC, nb], bf, tag="r_t")
                npre = act2.tile([128, HC, nb], f32, tag="npre")
                for m in range(HC):
                    col = m * 128
                    psz = psum.tile([128, nb], f32, tag="ps")
                    matseq_dr(psz, Wz, zr_kc, col, 128, rhs)
                    nc.scalar.activation(z_t[:, m, :], psz, AF.Sigmoid,
                                         bias=bz[:, m : m + 1], scale=1.0 / S)
                    psr = psum.tile([128, nb], f32, tag="ps")
                    matseq_dr(psr, Wr, zr_kc, col, 128, rhs)
                    nc.scalar.activation(r_t[:, m, :], psr, AF.Sigmoid,
                                         bias=br[:, m : m + 1], scale=1.0 / S)
                    psx = psum.tile([128, nb], f32, tag="ps")
                    matseq_dr(psx, Wnx, nx_kc, col, 128, rhs)
                    psh = psum.tile([128, nb], f32, tag="ps")
                    matseq_dr(psh, Wnh, HC, col, 128, rhs, r0=nh_r0)
                    # rhn = (psh + bnh) * r ; npre = (psx + bnx) + rhn
                    rhn = act3.tile([128, nb], f32, tag="rhn")
                    nc.vector.scalar_tensor_tensor(
                        rhn, psh, bnh[:, m : m + 1],
                        r_t[:, m, :], op0=ALU.add, op1=ALU.mult)
                    nc.vector.scalar_tensor_tensor(
                        npre[:, m, :], psx, bnx[:, m : m + 1],
                        rhn, op0=ALU.add, op1=ALU.add)
                    # one deferred blend op from the previous iteration rides
                    # along per chunk: its deps are long satisfied, so it is
                    # pure VectorE throughput here and never blocks the stt's
                    # behind it.
                    if fillers:
                        fillers.popleft()()
                # n = tanh(npre/S), one fat ACT over all 4 chunks
                n_t = act4.tile([128, HC, nb], bf, tag="n_t")
                nc.scalar.activation(n_t, npre, AF.Tanh, scale=1.0 / S)
                return z_t, n_t

            def blend(z_t, n_t, h_bl, out, full):
                """h' = n + z*(h - n) as 4 slot-sized closures interleaved into
                the next iteration's chunk loops. Chunk pairs are staggered so
                each closure's inputs are ready ~2us before it is reached: the
                GpSimd half-sub issued in slot 0 is consumed only in slot 2.
                full=True writes all 4 chunks of `out` (pads compute to zero);
                full=False (zr2op target) writes chunks 0..2 + rows 0..15 of
                chunk 3 so the h2 half of the operand is preserved."""
                d = act3.tile([128, HC, nb], bf, tag="d")
                zd = act3.tile([128, HC, nb], bf, tag="zd")

                def s0():
                    nc.gpsimd.tensor_sub(d[:, 0:2, :], h_bl[:, 0:2, :], n_t[:, 0:2, :])
                    nc.vector.tensor_sub(d[:, 2:4, :], h_bl[:, 2:4, :], n_t[:, 2:4, :])

                def s1():
                    nc.vector.tensor_mul(zd[:, 2:4, :], z_t[:, 2:4, :], d[:, 2:4, :])

                def s2():
                    if full:
                        nc.vector.tensor_add(out[:, 2:4, :], n_t[:, 2:4, :], zd[:, 2:4, :])
                    else:
                        nc.vector.tensor_add(out[:, 2, :], n_t[:, 2, :], zd[:, 2, :])
                        nc.vector.tensor_add(out[0:16, 3, :], n_t[0:16, 3, :],
                                             zd[0:16, 3, :])
                    nc.vector.tensor_mul(zd[:, 0:2, :], z_t[:, 0:2, :], d[:, 0:2, :])

                def s3():
                    nc.vector.tensor_add(out[:, 0:2, :], n_t[:, 0:2, :], zd[:, 0:2, :])

                return [s0, s1, s2, s3]

            def stage_gru1(t):
                sl = ts(t, nb)
                # zr2op = GRU2's [g1|h2] fp8 operand: g1 written by GRU1 blend,
                # h2 stitched in by DMA at concat rows 400..799.
                zr2op = io.tile([128, ZRC, nb], f8, tag="zr2op")
                nc.sync.dma_start(out=zr2op[16:128, 3, :], in_=h2_s0[:, sl])
                nc.sync.dma_start(out=zr2op[:, 4:7, :], in_=h2_s1[:, :, sl])
                ST[t]["zr2op"] = zr2op
                z_t, n_t = gru_mm(ST[t]["xh"], W["Wz1"], W["Wr1"], 6,
                                  W["Wn1x"], 2, W["Wn1h"], 2,
                                  BT["bz1"], BT["br1"], BT["bnx1"], BT["bnh1"])
                return blend(z_t, n_t, ST[t]["h1s"], zr2op, full=False)

            def stage_gru2(t):
                zr2op = ST[t]["zr2op"]
                g2 = io.tile([128, HC, nb], f8, tag="g2")
                z_t, n_t = gru_mm(zr2op, W["Wz2"], W["Wr2"], ZRC,
                                  W["Wn2x"], HC, W["Wn2h"], 3,
                                  BT["bz2"], BT["br2"], BT["bnx2"], BT["bnh2"])
                ST[t]["g2"] = g2
                return blend(z_t, n_t, ST[t]["h2s"], g2, full=True)

            def stage_fc2(t):
                g2 = ST[t].pop("g2")
                f2 = io.tile([128, FFp // 128, nb], f8, tag="f2")
                for m in range(FFp // 128):
                    ps = psum.tile([128, nb], f32, tag="ps")
                    matseq_dr(ps, W["Wfc2T"], HC, m * 128, 128, g2)
                    # f2 = relu(ps + S*b) = S*relu(W g2 + b)
                    nc.vector.tensor_scalar(
                        f2[:, m, :], ps, BT["bfc2"][:, m : m + 1], 0.0,
                        op0=ALU.add, op1=ALU.max)
                ST[t]["f2"] = f2

            def stage_fc3(t):
                f2 = ST[t].pop("f2")
                f3 = io.tile([128, FFp // 128, nb], f8, tag="f3")
                for m in range(FFp // 128):
                    ps = psum.tile([128, nb], f32, tag="ps")
                    matseq_dr(ps, W["Wfc3T"], FFp // 128, m * 128, 128, f2)
                    # f3 = relu(ps/S + S*b) = S*relu(W f2 + b)
                    nc.scalar.activation(f3[:, m, :], ps, AF.Relu,
                                         bias=BT["bfc3"][:, m : m + 1],
                                         scale=1.0 / S)
                ST[t]["f3"] = f3

            def stage_fc4(t):
                sl = ts(t, nb)
                f3 = ST[t].pop("f3")
                o = io.tile([128, Fp // 128, nb], bf, tag="o")
                for m in range(Fp // 128):
                    ps = psum.tile([128, nb], f32, tag="ps")
                    matseq_dr(ps, W["Wfc4T"], FFp // 128, m * 128, 128, f3)
                    nc.scalar.activation(o[:, m, :], ps, AF.Sigmoid,
                                         bias=BT["bfc4"][:, m : m + 1],
                                         scale=1.0 / (S * S))
                nc.sync.dma_start(out=outT_r[:, :, sl], in_=o)

            # startup loads: sync carries GRU1 weights + first inputs; GRU2
            # weights go on the (otherwise idle at start) gpsimd ring; FC
            # weights on the scalar ring (needed only from iteration 2 on).
            # Startup: only what iteration 0 needs loads immediately; GRU2 and
            # FC weights are pushed past the critical window so they do not
            # steal HBM bandwidth from xh/Wz1/Wr1 (first-matmul gating).
            load_first_inputs()
            load_w("Wz1", nc.sync, nc.scalar)
            load_bias()
            load_w("Wr1", nc.sync, nc.scalar)
            load_w("Wn1x", nc.sync)
            load_w("Wn1h", nc.scalar)
            load_blend_inputs(0)
            with tc.tile_wait_until(0.004):
                for name in ("Wz2", "Wr2", "Wn2x", "Wn2h"):
                    load_w(name, nc.gpsimd)
            with tc.tile_wait_until(0.009):
                for name in ("Wfc2T", "Wfc3T", "Wfc4T"):
                    load_w(name, nc.scalar)

            from collections import deque
            fillers = deque()
            PF = 1  # input prefetch depth (iterations ahead)
            for i in range(n_tiles + 4):
                nxt = []
                if i < n_tiles:
                    if i + PF < n_tiles:
                        load_inputs(i + PF)
                    nxt += stage_gru1(i)
                else:
                    # no A-stage to consume pending blends: flush them now so
                    # the B-stage below never reads a zr2op whose writes are
                    # still queued behind its own chain (circular wait).
                    while fillers:
                        fillers.popleft()()
                if 0 <= i - 1 < n_tiles:
                    nxt += stage_gru2(i - 1)
                if 0 <= i - 3 < n_tiles:
                    stage_fc3(i - 3)
                if 0 <= i - 4 < n_tiles:
                    stage_fc4(i - 4)
                if 0 <= i - 2 < n_tiles:
                    stage_fc2(i - 2)
                while fillers:
                    fillers.popleft()()
                fillers.extend(nxt)

    nc.compile()
    return nc


def _shard_inputs(inp, weights, biases):
    x = np.asarray(inp["x"], dtype=np.float32)
    h1 = np.asarray(inp["h1"], dtype=np.float32)
    h2 = np.asarray(inp["h2"], dtype=np.float32)

    xh8 = np.zeros((NCORES, XHK, BPC), dtype=FP8)    # matmul operand [x|h1]
    h1T = np.zeros((NCORES, Hp, BPC), dtype=BF16)    # blend h1
    h2T = np.zeros((NCORES, Hp, BPC), dtype=BF16)    # blend h2
    h28 = np.zeros((NCORES, Hp, BPC), dtype=FP8)     # zr2op h2 stitch source
    for i in range(NCORES):
        sl = slice(i * BPC, (i + 1) * BPC)
        xh8[i, :F] = x[sl].T.astype(FP8)
        xh8[i, F : F + H] = h1[sl].T.astype(FP8)
        h1T[i, :H] = h1[sl].T.astype(BF16)
        h2T[i, :H] = h2[sl].T.astype(BF16)
        h28[i, :H] = h2[sl].T.astype(FP8)

    in_maps = []
    for i in range(NCORES):
        m = {"xh8": xh8[i], "h1T": h1T[i], "h2T": h2T[i], "h28": h28[i]}
        m.update(weights)
        m.update(biases)
        in_maps.append(m)
    return in_maps


def _run(inp, trace=False):
    weights, biases = prepare_weights(inp)
    nc = build_nc()
    in_maps = _shard_inputs(inp, weights, biases)
    res = run_bass_kernel_spmd(nc, in_maps, list(range(NCORES)), trace=trace)
    out = np.empty((B, F), dtype=np.float32)
    for i in range(NCORES):
        out[i * BPC : (i + 1) * BPC] = (
            np.asarray(res.results[i]["outT"][:F]).astype(np.float32).T
        )
    return out, res


def kernel(**inputs) -> np.ndarray:
    out, _ = _run(inputs, trace=False)
    return out
